# revision 8
# baseline (speedup 1.0000x reference)
"""Bilateral filter (K=7, sigma_color=0.1) on 8 Trainium2 NeuronCores.

Odd-symmetry band-layout formulation:
    If = I + S'/W
    S'  = sum_{pairs o} k_o * [u_o(p+o) - u_o(p)],   u_o = h_o * d_o
    W   = g_c + sum_{pairs o} k_o * [h_o(p+o) + h_o(p)]
    d_o(x) = I(x) - I(x-o),  h_o = (2/sqrt(pi)) exp(-d^2/sc) via ACT D_ERF,
    k_o = g_o * sqrt(pi)/2.
Each opposite tap pair shares ONE subtract, ONE activation, ONE multiply
over a slightly-expanded flat domain; the center tap enters W via a
ones-tile matmul view.

Device mapping:
- 8 cores = 8 column bands of 80 cols (+3 halo each side -> 86).
- 128 partitions = 4 batches x 32 row-bands of 15 rows; each partition
  stores its 15 rows + 3-row halo as a flat [21*86]=1806 fp16 tile. All
  taps are pure flat-offset views; per-row "dead" halo columns flow
  through harmlessly and are skipped by the output DMA.
- 9 pair groups (consecutive flat deltas) merge into rank-2
  sub / D_ERF / mul ops (stride-0 in0, stride -1 in1).
- Engine balance: DVE does sub+mul (2x fp16 mode, 1.92 elem/ns); ACT
  does D_ERF (1.2 elem/ns) plus fp8 re-emissions of h for some pairs
  (halves those pairs' W matmul cost); GpSimd does pair presums of h
  (merging 2 W views into 1) and the small early muls; PE accumulates
  all views with +/-k scaled-identity weights into 6 PSUM banks.
- Epilogue: accW holds -(W); accS holds -S'. Per chunk: DVE
  reciprocal_approx_fast gives -1/W, one tensor_tensor mult gives
  S'/W as fp16. No ACT table swap, no Newton.
"""
import math

import numpy as np

import concourse.bacc as bacc
import concourse.tile as tile
from concourse import mybir
from concourse.bass_utils import run_bass_kernel_spmd
import bass_rust

K = 7
PAD = K // 2
H, W = 480, 640
N = 4
NCORES = 8
SIGMA_COLOR = 2.0 * 0.1 ** 2            # 0.02
CSC = 1.0 / math.sqrt(SIGMA_COLOR)      # DErf(d*CSC) = 2/sqrt(pi)*exp(-d^2/sc)
NT = K * K

P = 128                                  # partitions (4 batches x 32 bands)
BROWS = 15                               # output rows per partition
TR = BROWS + 2 * PAD                     # 21 stored rows
COLS = W // NCORES                       # 80 output cols per core
CW = COLS + 2 * PAD                      # 86 stored cols
AL = TR * CW                             # 1806 flat image elems
PMIN = PAD * CW + PAD                    # 261: flat index of first output px
FL = (BROWS - 1) * CW + COLS             # 1284: flat output span (w/ dead)

f32 = mybir.dt.float32
f16 = mybir.dt.float16
f8 = mybir.dt.float8e4

# groups of canonical pairs with consecutive Delta = 86*oy + ox
GSPECS = [(0, [1]), (0, [2, 3]),
          (1, [-3, -2, -1, 0]), (1, [1, 2, 3]),
          (2, [-3, -2, -1, 0]), (2, [1, 2, 3]),
          (3, [-3, -2, -1, 0]), (3, [2, 3]), (3, [1])]
_ALLP = [(oy, ox) for oy, oxs in GSPECS for ox in oxs]
_R2S = sorted({oy * oy + ox * ox for oy, ox in _ALLP})
NS = len(_R2S)                           # 9 radius classes

# pair-level strategy knobs
PRESUM = {(1, -2), (1, -1), (1, 0), (1, 1)}  # W pair-sum on DVE
H8 = set()                               # fp8 h re-emit on ACT
GMUL_G = set()                           # group indices with GpSimd mul
_R2S8 = sorted({oy * oy + ox * ox for oy, ox in H8})
NS8 = max(1, len(_R2S8))

_cache = {}


def _ap(base, off, dims):
    """Rank-(1+len(dims)) AP on base's tile at element offset off."""
    return bass_rust.AP(base.tensor, base.offset + off,
                        [list(base.ap[0])] + [list(d) for d in dims])


def _build_fast(g_center):
    nc = bacc.Bacc("TRN2", target_bir_lowering=False, debug=False,
                   num_devices=NCORES)
    a_ext = nc.declare_dram_parameter("a", [P, AL], f16, isOutput=False)
    eye_ext = nc.declare_dram_parameter("eye", [P, 2 * NS + 1, P], f16,
                                        isOutput=False)
    eye8_ext = nc.declare_dram_parameter("eye8", [P, NS8, P], f8,
                                         isOutput=False)
    o_ext = nc.declare_dram_parameter("o", [P, 1200], f16, isOutput=True)

    # PSUM banked-rows layout: bank b holds output rows 5b..5b+4 as a
    # 5x86-flat span (430 fp32 of 512); -(W) in banks 0-2, -S' in 3-5.
    NRCH = 3
    ROWS_PER = 5
    CCOLS = ROWS_PER * COLS                       # 400 dense cols per chunk

    groups = []
    for gi, (oy, ss) in enumerate(GSPECS):
        dmin, dmax = 86 * oy + ss[0], 86 * oy + ss[-1]
        groups.append(dict(gi=gi, oy=oy, oxs=ss, np=len(ss), dmin=dmin,
                           lbar=FL + dmax))

    # per-chunk matmul totals for start/stop flags
    def _wviews():
        n = 1                                     # gc ones view
        for oy, ox in _ALLP:
            if (oy, ox) in PRESUM:
                n += 1
            else:
                n += 2
        return n
    W_LAST = _wviews()
    S_LAST = 2 * len(_ALLP)

    with tile.TileContext(nc, pool_alloc_mode="queue") as tc:
        with tc.tile_pool(name="work", bufs=3) as pool, \
             tc.tile_pool(name="cst", bufs=1) as cpool, \
             tc.tile_pool(name="ep", bufs=1) as epool, \
             tc.tile_pool(name="ps", bufs=1, space="PSUM") as ppool:
            at = cpool.tile([P, AL], f16)
            eye_t = cpool.tile([P, 2 * NS + 1, P], f16)
            eye8_t = cpool.tile([P, NS8, P], f8)
            # HAM warmup: PE boots throttled and un-throttles only after
            # ~10us of sustained activity. Burn the idle preamble window
            # with garbage matmuls so the real stream starts warm. The
            # warmup tile doubles as the ones-tile for the gc view.
            wt = cpool.tile([P, 704], f16)
            nc.gpsimd.memset(wt, 1.0)
            wacc = ppool.tile([P, 512], f32, name="wacc")
            for _ in range(10):
                nc.tensor.matmul(wacc[:, 0:512], wt[:, 0:128],
                                 wt[:, 128:640], start=True, stop=True)
            nc.sync.dma_start(out=at[:, 172:1634], in_=a_ext[:, 172:1634])
            nc.gpsimd.dma_start(out=at[:, 0:172], in_=a_ext[:, 0:172])
            nc.gpsimd.dma_start(out=at[:, 1634:AL], in_=a_ext[:, 1634:AL])
            nc.sync.dma_start(out=eye_t[:, 0:NS, :], in_=eye_ext[:, 0:NS, :])
            nc.gpsimd.dma_start(out=eye_t[:, NS:2 * NS + 1, :],
                                in_=eye_ext[:, NS:2 * NS + 1, :])
            nc.sync.dma_start(out=eye8_t[:, :, :], in_=eye8_ext[:, :, :])

            accW = [ppool.tile([P, 512], f32, name=f"aw{i}")
                    for i in range(3)]
            accS = [ppool.tile([P, 512], f32, name=f"as{i}")
                    for i in range(3)]

            nW = [0, 0, 0]
            nS = [0, 0, 0]

            def emit_sub(g, po=None):
                po = cpool
                np_, lbar, dmin = g["np"], g["lbar"], g["dmin"]
                oy, ox0 = g["oy"], g["oxs"][0]
                nm = f"{oy}_{ox0}"
                dt = po.tile([P, np_, lbar], f16, name=f"d{nm}", tag=f"d{nm}")
                ht = po.tile([P, np_, lbar], f16, name=f"h{nm}", tag=f"h{nm}")
                in0 = _ap(at, PMIN, [[0, np_], [1, lbar]])
                in1 = _ap(at, PMIN - dmin, [[-1, np_], [1, lbar]])
                do = _ap(dt, 0, [[lbar, np_], [1, lbar]])
                nc.vector.tensor_tensor(do, in0, in1,
                                        mybir.AluOpType.subtract)
                nc.scalar.activation(ht[:, :, :], dt[:, :, :],
                                     mybir.ActivationFunctionType.
                                     Derivative_Erf, bias=0.0, scale=CSC)
                g["dt"], g["ht"] = dt, ht
                # fp8 re-emit of h for H8 pairs (contiguous j ranges only)
                js = [j for j, ox in enumerate(g["oxs"])
                      if (oy, ox) in H8]
                g["h8js"] = js
                if js:
                    j0, j1 = js[0], js[-1] + 1
                    assert js == list(range(j0, j1))
                    h8t = po.tile([P, j1 - j0, lbar], f8,
                                  name=f"h8{nm}", tag=f"h8{nm}")
                    nc.scalar.activation(h8t[:, :, :], dt[:, j0:j1, :],
                                         mybir.ActivationFunctionType.
                                         Derivative_Erf, bias=0.0, scale=CSC)
                    g["h8t"], g["h8j0"] = h8t, j0

            def emit_mul(g, po=None):
                np_, lbar = g["np"], g["lbar"]
                dt, ht = g["dt"], g["ht"]
                # u overwrites d in place: d is dead after h (and h8)
                nc.vector.tensor_tensor(dt[:, :, :], ht[:, :, :],
                                        dt[:, :, :], mybir.AluOpType.mult)
                g["ut"] = dt
                tfh = ht.rearrange("p a b -> p (a b)")
                g["hs"] = {}
                for j, ox in enumerate(g["oxs"]):
                    delta = 86 * oy + ox
                    if (oy, ox) in PRESUM:
                        ps_t = cpool.tile([P, FL], f16,
                                          name=f"c{oy}_{ox}",
                                          tag=f"c{oy}_{ox}")
                        nc.vector.tensor_tensor(
                            ps_t[:, :],
                            _ap(tfh, j * lbar + delta, [[1, FL]]),
                            _ap(tfh, j * lbar, [[1, FL]]),
                            mybir.AluOpType.add)
                        g["hs"][j] = ps_t

            def mm(bank, slot, vt, off, ci, nn, last, eye=None):
                rhs = _ap(vt, off + 86 * ROWS_PER * ci,
                          [[86, ROWS_PER], [1, COLS]])
                out = _ap(bank[ci], 0, [[86, ROWS_PER], [1, COLS]])
                et = eye if eye is not None else eye_t
                nc.tensor.matmul(out, et[:, slot, :], rhs,
                                 start=(nn[ci] == 0),
                                 stop=(nn[ci] == last - 1))
                nn[ci] += 1

            def w_views(g):
                """Yield (tensor_flat, offset, slot, eye_tile) W views."""
                oy, lbar = g["oy"], g["lbar"]
                tf = g["ht"].rearrange("p a b -> p (a b)")
                tf8 = g["h8t"].rearrange("p a b -> p (a b)") \
                    if g.get("h8js") else None
                out = []
                for j, ox in enumerate(g["oxs"]):
                    delta = 86 * oy + ox
                    s = _R2S.index(oy * oy + ox * ox)
                    if j in g["hs"]:
                        out.append((g["hs"][j], 0, s, None))
                    elif (oy, ox) in H8:
                        s8 = _R2S8.index(oy * oy + ox * ox)
                        j8 = j - g["h8j0"]
                        out.append((tf8, j8 * lbar + delta, s8, eye8_t))
                        out.append((tf8, j8 * lbar, s8, eye8_t))
                    else:
                        out.append((tf, j * lbar + delta, s, None))
                        out.append((tf, j * lbar, s, None))
                return out

            def s_views(g):
                oy, lbar = g["oy"], g["lbar"]
                tf = g["ut"].rearrange("p a b -> p (a b)")
                out = []
                for j, ox in enumerate(g["oxs"]):
                    delta = 86 * oy + ox
                    s = _R2S.index(oy * oy + ox * ox)
                    out.append((tf, j * lbar + delta, s, None))
                    out.append((tf, j * lbar, NS + s, None))
                return out

            # epilogue tiles (dense 1200)
            r_t = epool.tile([P, 1200], f32)
            of = epool.tile([P, 1200], f16)

            def acc_ap(bank, ci, r0=0, r1=ROWS_PER):
                return _ap(bank[ci], 86 * r0, [[86, r1 - r0], [1, COLS]])

            def dn(tile_, ci, r0=0, r1=ROWS_PER):
                return _ap(tile_, CCOLS * ci + COLS * r0,
                           [[COLS, r1 - r0], [1, COLS]])

            def emit_gc(ci):
                # center tap: -gc * ones into accW (wt is all-ones; the
                # view is chunk-independent, always offset 0)
                rhs = _ap(wt, 0, [[86, ROWS_PER], [1, COLS]])
                out = _ap(accW[ci], 0, [[86, ROWS_PER], [1, COLS]])
                nc.tensor.matmul(out, eye_t[:, 2 * NS, :], rhs,
                                 start=(nW[ci] == 0),
                                 stop=(nW[ci] == W_LAST - 1))
                nW[ci] += 1

            def emit_tail_chunk(ci):
                # r = -1/W; of = (-S')*(-1/W) = S'/W; split the last
                # chunk so the final mul/dma cascade is half as deep
                nc.vector.reciprocal_approx_fast(
                    dn(r_t, ci), acc_ap(accW, ci))
                parts = ((0, ROWS_PER),) if ci < NRCH - 1 else \
                    ((0, 3), (3, ROWS_PER))
                for (r0, r1) in parts:
                    nc.vector.tensor_tensor(
                        dn(of, ci, r0, r1), acc_ap(accS, ci, r0, r1),
                        dn(r_t, ci, r0, r1), mybir.AluOpType.mult)
                    c0 = CCOLS * ci + COLS * r0
                    c1 = CCOLS * ci + COLS * r1
                    eng = nc.sync if (ci + r0) % 2 == 0 else nc.gpsimd
                    eng.dma_start(out=o_ext[:, c0:c1], in_=of[:, c0:c1])

            def emit_W(g):
                for (vt, off, slot, eye) in w_views(g):
                    for ci in range(NRCH):
                        mm(accW, slot, vt, off, ci, nW, W_LAST, eye)

            def emit_S(g):
                for (vt, off, slot, eye) in s_views(g):
                    for ci in range(NRCH):
                        mm(accS, slot, vt, off, ci, nS, S_LAST, eye)

            g0, gS = groups[0], groups[1]
            rest = groups[2:]
            emit_sub(g0)
            emit_sub(gS, cpool)
            emit_mul(g0)
            for ci in range(NRCH):
                emit_gc(ci)
            emit_W(g0)
            emit_S(g0)
            emit_mul(gS, cpool)
            emit_W(gS)
            emit_S(gS)
            emit_sub(rest[0])
            for i, g in enumerate(rest):
                if i + 1 < len(rest):
                    emit_sub(rest[i + 1])
                emit_mul(g)
                emit_W(g)
                if i < len(rest) - 2:
                    emit_S(g)
            emit_S(rest[-2])
            # last group: per-chunk S tail overlapped with epilogue
            sviews = s_views(rest[-1])
            for ci in range(NRCH):
                for (vt, off, slot, eye) in sviews:
                    mm(accS, slot, vt, off, ci, nS, S_LAST, eye)
                emit_tail_chunk(ci)
    nc.compile()
    return nc


def _get_nc(fast):
    assert fast, "fallback path is numpy-only"
    gc = _cache["g_center"]
    if _cache.get("fast_gc") != gc:
        _cache["fast"] = _build_fast(gc)
        _cache["fast_gc"] = gc
    return _cache["fast"]


def _shard_image(I):
    """I: (N,1,H,W) f32 -> per-core [P, AL] fp16 tiles."""
    Ip = np.zeros((N, H + 2 * PAD, W + 2 * PAD), np.float16)
    Ip[:, PAD:PAD + H, PAD:PAD + W] = I[:, 0]
    shards = []
    s0, s1 = Ip.strides[1], Ip.strides[2]
    for c in range(NCORES):
        blk = Ip[:, :, COLS * c:COLS * c + CW]       # (4, 486, 86)
        bands = np.lib.stride_tricks.as_strided(
            blk, shape=(N, 32, TR, CW),
            strides=(Ip.strides[0], BROWS * s0, s0, s1))
        shards.append(np.ascontiguousarray(bands).reshape(P, AL))
    return shards


def _eye(gdict, g_center):
    import ml_dtypes
    eye = np.zeros((P, 2 * NS + 1, P), np.float32)
    idx = np.arange(P)
    for s, r2 in enumerate(_R2S):
        k = gdict[r2] * math.sqrt(math.pi) / 2.0
        eye[idx, s, idx] = -k
        eye[idx, NS + s, idx] = k
    eye[idx, 2 * NS, idx] = -g_center
    eye8 = np.zeros((P, NS8, P), np.float32)
    for s, r2 in enumerate(_R2S8):
        k = gdict[r2] * math.sqrt(math.pi) / 2.0
        eye8[idx, s, idx] = -k
    return (eye.astype(np.float16),
            eye8.astype(ml_dtypes.float8_e4m3fn))


def _prepare(I, g):
    I = np.ascontiguousarray(np.asarray(I, dtype=np.float32))
    g = np.asarray(g, dtype=np.float32)
    gs = g[0, :, 0, 0]
    fast = bool(np.array_equal(
        g, np.broadcast_to(gs[None, :, None, None], g.shape))) and bool(
        np.all(gs > 0))
    if not fast:
        return False, None
    gdict = {}
    ok = True
    for t in range(NT):
        r2 = (t // K - PAD) ** 2 + (t % K - PAD) ** 2
        if r2 in gdict:
            ok = ok and abs(gdict[r2] - float(gs[t])) <= 1e-6 * abs(gdict[r2])
        else:
            gdict[r2] = float(gs[t])
    if not ok:
        return False, None
    _cache["g_center"] = gdict[0]
    eye, eye8 = _eye(gdict, gdict[0])
    in_maps = [{"a": a, "eye": eye, "eye8": eye8} for a in _shard_image(I)]
    return True, in_maps


def _numpy_fallback(I, g):
    I64 = np.asarray(I, np.float64)
    Ip = np.pad(I64[:, 0], ((0, 0), (PAD, PAD), (PAD, PAD)))
    out_w = np.zeros((N, H, W))
    out_s = np.zeros((N, H, W))
    g64 = np.asarray(g, np.float64)
    for t in range(NT):
        y, x = divmod(t, K)
        tap = Ip[:, y:y + H, x:x + W]
        d = tap - I64[:, 0]
        e = np.exp(-d * d / SIGMA_COLOR) * g64[:, t]
        out_w += e
        out_s += e * tap
    return (out_s / out_w).astype(np.float32)


def kernel(I, g):
    fast, in_maps = _prepare(I, g)
    if not fast:
        return _numpy_fallback(I, g)
    nc = _get_nc(True)
    try:
        res = run_bass_kernel_spmd(nc, in_maps, list(range(NCORES)))
    except Exception:
        res = run_bass_kernel_spmd(nc, in_maps, list(range(NCORES)))
    out = np.empty((N, H, W), np.float32)
    I32 = np.asarray(I, dtype=np.float32)
    for c in range(NCORES):
        o = res.results[c]["o"]                      # [P, 1200] f16: S'/W
        out[:, :, COLS * c:COLS * (c + 1)] = (
            o.reshape(N, H, COLS).astype(np.float32)
            + I32[:, 0, :, COLS * c:COLS * (c + 1)])
    return out
